# revision 16
# baseline (speedup 1.0000x reference)
"""GRAFF GNN kernel for Trainium2, 8 NeuronCores, SPMD.

Feature-major ("T-layout") design: node state lives as hT [128 feat, 2 fb x
12544 nodes] bf16 in SBUF, which makes every matmul in the network a direct
lhsT access with zero on-device transposes:
  - encoder:  hT = enc_wT^T @ xT          (lhsT = enc_w^T k-tiles, rhs = xT)
  - out_p:    po  = hT^T @ Wp             (lhsT = hT, Wp symmetric)
  - scatter:  aggT = msg^T @ S            (lhsT = gathered msg rows)
  - MLP:      gT = W1 @ hT, yT = W2 @ gT  (lhsT = W1^T / W2^T k-tiles)
BatchNorm becomes per-partition scale/bias fused into one activation op per
feature block. The segment-sum scatter matmuls use host-precomputed one-hot
S matrices (norm folded in) streamed from DRAM; edge-message gathers are one
batched indirect DMA per 7-window chunk. Nodes split 8 ways; edges
partitioned by destination; per-layer AllGather of node-major out_p bf16.

Self-contained: hardcodes shapes from the problem spec.
"""
import sys
sys.path.insert(0, "/opt/trn_rl_repo")
import numpy as np
import ml_dtypes

import os
import concourse.bass as bass
import concourse.bacc as bacc
import concourse.tile as tile
from concourse import mybir
from concourse.bass_utils import run_bass_kernel_spmd
from contextlib import ExitStack

bf16 = ml_dtypes.bfloat16
FP32 = mybir.dt.float32
BF16 = mybir.dt.bfloat16
I32 = mybir.dt.int32

NCORE = 8
N = 100000
FIN = 2613
H = 256
NL = 12500            # real nodes per core
NW = 98               # windows of 128 dests (last has 84 real)
NSH = NW * 128        # padded shard rows = 12544
KT = 21               # k-tiles of 128 over FIN (padded to 2688)
FINP = KT * 128
STEP = 0.1
CH = 7                # windows per chunk (98 = 14*7)
NCH = NW // CH        # 14 chunks
NB = 25               # encoder/MLP node blocks per core (24x512 + 212)
RG = [list(range(NCORE))]
ACT_FN = mybir.ActivationFunctionType


def _nb_range(b):
    n0 = b * 512
    return n0, min(512, NL - n0)


def _build_program(K_w, src_b_val):
    """Build the SPMD Bass program. K_w: list of e-tile counts per window."""
    K_w = [int(k) for k in K_w]
    n_et = int(sum(K_w))
    etb = np.concatenate([[0], np.cumsum(K_w)]).astype(int)
    KC = [int(etb[(c + 1) * CH] - etb[c * CH]) for c in range(NCH)]
    KCmax = max(KC)

    nc = bacc.Bacc("TRN2", num_devices=NCORE, debug=False)

    # ---- I/O ----
    xt_t = nc.dram_tensor("xt", [FINP, NL], BF16, kind="ExternalInput")
    eidx_t = nc.dram_tensor("eidx", [128, n_et], I32, kind="ExternalInput")
    s_t = nc.dram_tensor("smat", [128, n_et * 128], BF16, kind="ExternalInput")
    encw_t = nc.dram_tensor("encw", [128, KT * H], BF16, kind="ExternalInput")
    wp_t = nc.dram_tensor("wp", [128, 2 * H], BF16, kind="ExternalInput")
    l1_t = nc.dram_tensor("l1", [128, 2 * H], BF16, kind="ExternalInput")
    l2_t = nc.dram_tensor("l2", [128, 2 * H], BF16, kind="ExternalInput")
    # per-partition (feature) columns: [-ext_w | b1 | b2 | gamma | beta]
    cols_t = nc.dram_tensor("cols", [128, 10], FP32, kind="ExternalInput")
    y_t = nc.dram_tensor("y", [2 * 128, NL], FP32, kind="ExternalOutput")

    # collectives (internal DRAM)
    hsh = [nc.dram_tensor(f"hsh{i}", [NSH, H], BF16, kind="Internal")
           for i in range(4)]
    hfull = [nc.dram_tensor(f"hfull{i}", [NCORE * NSH, H], BF16,
                            kind="Internal", addr_space="Shared")
             for i in range(4)]
    h0d = nc.dram_tensor("h0d", [128, 2 * NSH], BF16, kind="Internal")
    ar_in = nc.dram_tensor("ar_in", [128, 4], FP32, kind="Internal")
    ar_out = nc.dram_tensor("ar_out", [128, 4], FP32, kind="Internal",
                            addr_space="Shared")

    with tile.TileContext(nc) as tc, ExitStack() as ctx:
        const = ctx.enter_context(tc.tile_pool(name="const", bufs=1))
        xtp = ctx.enter_context(tc.tile_pool(name="xtp", bufs=3))
        big = ctx.enter_context(tc.tile_pool(name="big", bufs=2))
        sp = ctx.enter_context(tc.tile_pool(name="sp", bufs=2))
        sb = ctx.enter_context(tc.tile_pool(name="sb", bufs=2))
        up = ctx.enter_context(tc.tile_pool(name="up", bufs=1))
        ep = ctx.enter_context(tc.tile_pool(name="ep", bufs=2))
        ps_agg = ctx.enter_context(tc.tile_pool(name="psa", bufs=1, space="PSUM"))
        ps_po = ctx.enter_context(tc.tile_pool(name="psp", bufs=2, space="PSUM"))
        ps_blk = ctx.enter_context(tc.tile_pool(name="psb", bufs=2, space="PSUM"))

        # ---- resident constants / state ----
        hT = const.tile([128, 2 * NSH], BF16)      # state, [p, fb*NSH + n]
        eidx_sb = const.tile([128, n_et], I32)
        nc.sync.dma_start(out=eidx_sb[:], in_=eidx_t.ap())
        encw_sb = const.tile([128, KT * H], BF16)
        nc.sync.dma_start(out=encw_sb[:], in_=encw_t.ap())
        wp_sb = const.tile([128, 2 * H], BF16)
        nc.sync.dma_start(out=wp_sb[:], in_=wp_t.ap())
        l1_sb = const.tile([128, 2 * H], BF16)
        nc.sync.dma_start(out=l1_sb[:], in_=l1_t.ap())
        l2_sb = const.tile([128, 2 * H], BF16)
        nc.sync.dma_start(out=l2_sb[:], in_=l2_t.ap())
        cols_sb = const.tile([128, 10], FP32)
        nc.sync.dma_start(out=cols_sb[:], in_=cols_t.ap())
        extw_neg = cols_sb[:, 0:2]
        b1_c = cols_sb[:, 2:4]
        b2_c = cols_sb[:, 4:6]
        gam_c = cols_sb[:, 6:8]
        bet_c = cols_sb[:, 8:10]

        acc = const.tile([128, 4 * NB], FP32)      # BN stat cols
        # zero the padded node columns once (12500..12544 of each fb half)
        for fb in range(2):
            nc.vector.memset(hT[:, fb * NSH + NL: (fb + 1) * NSH], 0.0)

        # ================= encoder (pre-BN hT + stats) =================
        KT_SPLIT = [(0, 11), (11, 10)]             # halve SBUF for xt tiles
        for b in range(NB):
            n0, nn_ = _nb_range(b)
            xts_ = []
            src = xt_t.ap().rearrange("(kt p) n -> p kt n", p=128)
            for (k0, nk) in KT_SPLIT:
                xt = xtp.tile([128, 11 * 512], BF16, tag="xt")
                nc.sync.dma_start(
                    out=xt[:].rearrange("p (kt n) -> p kt n", kt=11)[:, :nk, :nn_],
                    in_=src[:, k0: k0 + nk, n0: n0 + nn_])
                xts_.append(xt)
            for fb in range(2):
                ph = ps_blk.tile([128, 512], FP32, tag="blk", space="PSUM")
                for kt in range(KT):
                    half, klo = (0, kt) if kt < 11 else (1, kt - 11)
                    nc.tensor.matmul(
                        ph[:, :nn_],
                        lhsT=encw_sb[:, kt * H + fb * 128: kt * H + fb * 128 + 128],
                        rhs=xts_[half][:, klo * 512: klo * 512 + nn_],
                        start=(kt == 0), stop=(kt == KT - 1))
                # PSUM -> bf16 state, accumulate row-sums for BN stats
                nc.scalar.activation(
                    hT[:, fb * NSH + n0: fb * NSH + n0 + nn_], ph[:, :nn_],
                    ACT_FN.Copy, accum_out=acc[:, 4 * b + fb: 4 * b + fb + 1])
                sq = ep.tile([128, 512], BF16, tag="sq")
                nc.scalar.activation(
                    sq[:, :nn_], ph[:, :nn_], ACT_FN.Square,
                    accum_out=acc[:, 4 * b + 2 + fb: 4 * b + 3 + fb])

        # ================= batch norm =================
        st = sb.tile([128, 4], FP32, tag="st")
        for j in range(4):
            nc.vector.tensor_reduce(
                out=st[:, j: j + 1],
                in_=acc[:].rearrange("p (b j) -> p j b", j=4)[:, j, :],
                axis=mybir.AxisListType.X, op=mybir.AluOpType.add)
        nc.sync.dma_start(out=ar_in.ap(), in_=st[:])
        if os.environ.get("K_NOAG"):
            nc.sync.dma_start(out=ar_out.ap(), in_=ar_in.ap())
        else:
            nc.gpsimd.collective_compute(
                "AllReduce", mybir.AluOpType.add, replica_groups=RG,
                ins=[ar_in.ap()], outs=[ar_out.ap()])
        sg = sb.tile([128, 4], FP32, tag="sg")
        nc.sync.dma_start(out=sg[:], in_=ar_out.ap())

        mean = sb.tile([128, 2], FP32, tag="mean")
        nc.vector.tensor_scalar(out=mean[:], in0=sg[:, 0:2], scalar1=1.0 / N,
                                scalar2=None, op0=mybir.AluOpType.mult)
        var = sb.tile([128, 2], FP32, tag="var")
        nc.vector.tensor_scalar(out=var[:], in0=sg[:, 2:4], scalar1=1.0 / N,
                                scalar2=None, op0=mybir.AluOpType.mult)
        msq = sb.tile([128, 2], FP32, tag="msq")
        nc.vector.tensor_mul(msq[:], mean[:], mean[:])
        nc.vector.tensor_sub(var[:], var[:], msq[:])
        nc.vector.tensor_scalar(out=var[:], in0=var[:], scalar1=1e-5,
                                scalar2=None, op0=mybir.AluOpType.add)
        std = sb.tile([128, 2], FP32, tag="std")
        nc.scalar.activation(std[:], var[:], ACT_FN.Sqrt)
        inv = sb.tile([128, 2], FP32, tag="inv")
        nc.vector.reciprocal(inv[:], std[:])
        scl = sb.tile([128, 2], FP32, tag="scl")
        nc.vector.tensor_mul(scl[:], inv[:], gam_c)
        shf = sb.tile([128, 2], FP32, tag="shf")
        nc.vector.tensor_mul(shf[:], mean[:], scl[:])
        nc.vector.tensor_sub(shf[:], bet_c, shf[:])
        for fb in range(2):
            hv = hT[:, fb * NSH: fb * NSH + NSH]
            nc.vector.scalar_tensor_tensor(
                out=hv, in0=hv, scalar=scl[:, fb: fb + 1],
                in1=shf[:, fb: fb + 1].to_broadcast([128, NSH]),
                op0=mybir.AluOpType.mult, op1=mybir.AluOpType.add)
        nc.sync.dma_start(out=h0d.ap(), in_=hT[:])  # h0 snapshot to DRAM

        # ---- out_p staging: po = h @ Wp (node-major) for chunk wc ----
        def stage_outp(li, wc):
            po_sb = sp.tile([128, CH * H], BF16, tag="po_sb")
            for wi in range(CH):
                w = wc * CH + wi
                po = ps_po.tile([128, 512], FP32, tag="po", space="PSUM")
                for fb in range(2):
                    nc.tensor.matmul(
                        po[:, :H],
                        lhsT=hT[:, fb * NSH + w * 128: fb * NSH + (w + 1) * 128],
                        rhs=wp_sb[:, fb * H: (fb + 1) * H],
                        start=(fb == 0), stop=(fb == 1))
                nc.scalar.activation(po_sb[:, wi * H: (wi + 1) * H],
                                     po[:, :H], ACT_FN.Copy)
            nc.sync.dma_start(
                out=hsh[li].ap().rearrange("(w p) f -> p w f", p=128)
                [:, wc * CH: (wc + 1) * CH, :],
                in_=po_sb[:].rearrange("p (w f) -> p w f", f=H))

        def allgather(li):
            if os.environ.get("K_NOAG"):
                nc.sync.dma_start(out=hfull[li].ap()[:NSH, :], in_=hsh[li].ap())
            else:
                nc.gpsimd.collective_compute(
                    "AllGather", mybir.AluOpType.bypass, replica_groups=RG,
                    ins=[hsh[li].ap()], outs=[hfull[li].ap()])

        for wc in range(NCH):
            stage_outp(0, wc)
        allgather(0)

        # ================= GNN layers =================
        _nlayers = int(os.environ.get("K_NLAYERS", "4"))
        for li in range(_nlayers):
            hf = hfull[li].ap()
            for wc in range(NCH):
                c0, c1 = int(etb[wc * CH]), int(etb[(wc + 1) * CH])
                kc = c1 - c0
                # batched gather of out_p rows for all e-tiles of the chunk
                msg = big.tile([128, KCmax * H], BF16, tag="big")
                gmode = os.environ.get("K_G", "tile")
                if os.environ.get("K_NOGATHER"):
                    nc.sync.dma_start(
                        out=msg[:, :kc * H].rearrange("p (k h) -> p k h", k=kc),
                        in_=hf.rearrange("(k p) f -> p k f", p=128)[:, :kc, :])
                elif gmode == "tile":
                    for j in range(kc):
                        nc.gpsimd.indirect_dma_start(
                            out=msg[:, j * H: (j + 1) * H],
                            out_offset=None, in_=hf,
                            in_offset=bass.IndirectOffsetOnAxis(
                                ap=eidx_sb[:, c0 + j: c0 + j + 1], axis=0))
                else:
                    nsplit = {"chunk": 1, "half": 2}.get(gmode, 1)
                    bnds = [kc * s // nsplit for s in range(nsplit + 1)]
                    for s in range(nsplit):
                        j0, j1 = bnds[s], bnds[s + 1]
                        nc.gpsimd.indirect_dma_start(
                            out=msg[:, j0 * H: j1 * H].rearrange(
                                "p (k h) -> p k h", k=j1 - j0),
                            out_offset=None, in_=hf,
                            in_offset=bass.IndirectOffsetOnAxis(
                                ap=eidx_sb[:, c0 + j0: c0 + j1], axis=0))
                S_t = sp.tile([128, KCmax * 128], BF16, tag="S")
                nc.sync.dma_start(out=S_t[:, :kc * 128],
                                  in_=s_t.ap()[:, c0 * 128: c1 * 128])

                # one accumulation group per 2KB PSUM bank (= 2 windows):
                # start zeroes the whole bank, stop closes it, chains in
                # between accumulate into disjoint 128-col regions.
                pagg = ps_agg.tile([128, CH * H], FP32, tag="agg", space="PSUM")
                for b in range((CH + 1) // 2):
                    chains = [(wi, fb) for wi in (2 * b, 2 * b + 1)
                              if wi < CH for fb in range(2)]
                    for ci, (wi, fb) in enumerate(chains):
                        w = wc * CH + wi
                        kw = K_w[w]
                        for t in range(kw):
                            et = int(etb[w]) - c0 + t
                            nc.tensor.matmul(
                                pagg[:, wi * H + fb * 128: wi * H + fb * 128 + 128],
                                lhsT=msg[:, et * H + fb * 128: et * H + fb * 128 + 128],
                                rhs=S_t[:, et * 128: (et + 1) * 128],
                                start=(ci == 0 and t == 0),
                                stop=(ci == len(chains) - 1 and t == kw - 1))

                # u = aggT - h*ext_w - src_b*h0   (T-layout, whole chunk)
                u_t = up.tile([128, 2 * CH * 128], FP32, tag="u")
                h0c = sp.tile([128, 2 * CH * 128], BF16, tag="h0c")
                nsl = slice(wc * CH * 128, (wc + 1) * CH * 128)
                nc.sync.dma_start(
                    out=h0c[:].rearrange("p (g m) -> p g m", g=2),
                    in_=h0d.ap().rearrange("p (g n) -> p g n", g=2)[:, :, nsl])
                for fb in range(2):
                    uv = u_t[:, fb * CH * 128: (fb + 1) * CH * 128].rearrange(
                        "p (w d) -> p w d", d=128)
                    av = pagg[:].rearrange("p (w g d) -> p w g d", g=2, d=128)[:, :, fb, :]
                    hv = hT[:, fb * NSH: (fb + 1) * NSH][:, nsl].rearrange(
                        "p (w d) -> p w d", d=128)
                    h0v = h0c[:, fb * CH * 128: (fb + 1) * CH * 128].rearrange(
                        "p (w d) -> p w d", d=128)
                    nc.vector.scalar_tensor_tensor(
                        out=uv, in0=hv, scalar=extw_neg[:, fb: fb + 1], in1=av,
                        op0=mybir.AluOpType.mult, op1=mybir.AluOpType.add)
                    nc.vector.scalar_tensor_tensor(
                        out=uv, in0=h0v, scalar=-src_b_val, in1=uv,
                        op0=mybir.AluOpType.mult, op1=mybir.AluOpType.add)
                # h += 0.1*elu(u); elu = relu(u) - relu(1-exp(u))
                e_t = ep.tile([128, 2 * CH * 128], BF16, tag="e")
                nc.scalar.activation(e_t[:], u_t[:], ACT_FN.Exp)
                a_t = ep.tile([128, 2 * CH * 128], BF16, tag="a")
                nc.scalar.activation(a_t[:], u_t[:], ACT_FN.Relu, scale=STEP)
                nc.vector.tensor_scalar(out=e_t[:], in0=e_t[:], scalar1=-STEP,
                                        scalar2=STEP, op0=mybir.AluOpType.mult,
                                        op1=mybir.AluOpType.add)
                nc.vector.tensor_scalar(out=e_t[:], in0=e_t[:], scalar1=0.0,
                                        scalar2=None, op0=mybir.AluOpType.max)
                nc.vector.tensor_sub(a_t[:], a_t[:], e_t[:])
                for fb in range(2):
                    hv = hT[:, fb * NSH: (fb + 1) * NSH][:, nsl]
                    nc.vector.tensor_add(
                        hv, hv, a_t[:, fb * CH * 128: (fb + 1) * CH * 128])
                if li < 3:
                    stage_outp(li + 1, wc)
            if li < 3:
                allgather(li + 1)

        # ================= MLP =================
        if os.environ.get("K_DBG"):
            # dump hT (bf16 state) to y for debugging: y[fo] rows = hT fb
            for fb in range(2):
                for b in range(NB):
                    n0, nn_ = _nb_range(b)
                    t = sb.tile([128, 512], FP32, tag="ysb")
                    nc.vector.tensor_copy(
                        out=t[:, :nn_],
                        in_=hT[:, fb * NSH + n0: fb * NSH + n0 + nn_])
                    nc.sync.dma_start(
                        out=y_t.ap()[fb * 128: (fb + 1) * 128, n0: n0 + nn_],
                        in_=t[:, :nn_])
        if not os.environ.get("K_NOMLP") and not os.environ.get("K_DBG"):
            for b in range(NB):
                n0, nn_ = _nb_range(b)
                gT = sb.tile([128, 2 * 512], BF16, tag="gT")
                for fo in range(2):
                    pg = ps_blk.tile([128, 512], FP32, tag="blk", space="PSUM")
                    for fb in range(2):
                        nc.tensor.matmul(
                            pg[:, :nn_],
                            lhsT=l1_sb[:, fb * H + fo * 128: fb * H + fo * 128 + 128],
                            rhs=hT[:, fb * NSH + n0: fb * NSH + n0 + nn_],
                            start=(fb == 0), stop=(fb == 1))
                    # g = elu(pg + b1)
                    e_m = ep.tile([128, 512], BF16, tag="sq")
                    nc.scalar.activation(e_m[:, :nn_], pg[:, :nn_], ACT_FN.Exp,
                                         bias=b1_c[:, fo: fo + 1])
                    nc.scalar.activation(gT[:, fo * 512: fo * 512 + nn_],
                                         pg[:, :nn_], ACT_FN.Relu,
                                         bias=b1_c[:, fo: fo + 1])
                    nc.vector.tensor_scalar(
                        out=e_m[:, :nn_], in0=e_m[:, :nn_], scalar1=-1.0,
                        scalar2=1.0, op0=mybir.AluOpType.mult,
                        op1=mybir.AluOpType.add)
                    nc.vector.tensor_scalar(
                        out=e_m[:, :nn_], in0=e_m[:, :nn_], scalar1=0.0,
                        scalar2=None, op0=mybir.AluOpType.max)
                    gv = gT[:, fo * 512: fo * 512 + nn_]
                    nc.vector.tensor_sub(gv, gv, e_m[:, :nn_])
                for fo in range(2):
                    py = ps_po.tile([128, 512], FP32, tag="po", space="PSUM")
                    for fb in range(2):
                        nc.tensor.matmul(
                            py[:, :nn_],
                            lhsT=l2_sb[:, fb * H + fo * 128: fb * H + fo * 128 + 128],
                            rhs=gT[:, fb * 512: fb * 512 + nn_],
                            start=(fb == 0), stop=(fb == 1))
                    y_sb = sb.tile([128, 512], FP32, tag="ysb")
                    nc.vector.tensor_add(
                        y_sb[:, :nn_], py[:, :nn_],
                        b2_c[:, fo: fo + 1].to_broadcast([128, nn_]))
                    nc.sync.dma_start(
                        out=y_t.ap()[fo * 128: (fo + 1) * 128, n0: n0 + nn_],
                        in_=y_sb[:, :nn_])

    nc.compile()
    return nc


def _host_prep(x, edge_index, enc_w, bn_gamma, bn_beta, ext_w, src_b, pw_W,
               lin1_w, lin1_b, lin2_w, lin2_b):
    x = np.asarray(x, dtype=np.float32)
    ei = np.asarray(edge_index)
    row = ei[0].astype(np.int64)
    col = ei[1].astype(np.int64)

    # pairwise matrix
    pw = np.asarray(pw_W, dtype=np.float32)
    W0 = np.triu(pw[:, :-2], k=1)
    W0 = W0 + W0.T
    Wp = W0 + np.diag(pw[:, -2] * np.abs(W0).sum(1) + pw[:, -1])

    deg = np.bincount(col, minlength=N).astype(np.float32)
    with np.errstate(divide="ignore"):
        dinv = np.where(deg > 0, deg ** -0.5, 0.0).astype(np.float32)
    nrm = (dinv[row] * dinv[col]).astype(np.float32)

    order = np.argsort(col, kind="stable")
    row_s, col_s, nrm_s = row[order], col[order], nrm[order]
    core_s = col_s // NL
    wloc_s = (col_s % NL) // 128
    dloc_s = (col_s % NL - wloc_s * 128).astype(np.int64)
    rowg_s = ((row_s // NL) * NSH + row_s % NL).astype(np.int32)

    counts = np.zeros((NCORE, NW), dtype=np.int64)
    np.add.at(counts, (core_s, wloc_s), 1)
    K_w = np.maximum(1, np.ceil(counts.max(0) / 128).astype(np.int64))
    n_et = int(K_w.sum())
    etb = np.concatenate([[0], np.cumsum(K_w)]).astype(np.int64)

    # slot of each edge inside its (core, window) block
    wid = core_s * NW + wloc_s
    start_of_block = np.zeros(NCORE * NW + 1, dtype=np.int64)
    np.add.at(start_of_block[1:], wid, 1)
    start_of_block = np.cumsum(start_of_block)
    rank = np.arange(len(col_s)) - start_of_block[wid]
    slot = etb[wloc_s] * 128 + rank  # within-core padded edge slot

    eidxs, smats = [], []
    for c in range(NCORE):
        m = core_s == c
        eidx_pad = np.zeros(n_et * 128, dtype=np.int32)
        eidx_pad[slot[m]] = rowg_s[m]
        eidx_T = np.ascontiguousarray(eidx_pad.reshape(n_et, 128).T)
        # one-hot scatter matrices with norm folded in: [slot, dest] -> nrm
        S_flat = np.zeros((n_et * 128, 128), dtype=np.float32)
        S_flat[slot[m], dloc_s[m]] = nrm_s[m]
        S_all = np.ascontiguousarray(
            S_flat.reshape(n_et, 128, 128).transpose(1, 0, 2)
        ).reshape(128, n_et * 128).astype(bf16)
        eidxs.append(eidx_T)
        smats.append(S_all)

    # per-core padded x^T in bf16
    xts = []
    for c in range(NCORE):
        xt = np.zeros((FINP, NL), dtype=bf16)
        xt[:FIN] = np.ascontiguousarray(x[c * NL:(c + 1) * NL].T).astype(bf16)
        xts.append(xt)

    def ktile_layout(mat_T, nk):  # mat_T [nk*128, H] -> [128, nk*H]
        out = np.zeros((128, nk * H), dtype=bf16)
        for kt in range(nk):
            blk = mat_T[kt * 128:(kt + 1) * 128]
            out[:blk.shape[0], kt * H:kt * H + blk.shape[1]] = blk.astype(bf16)
        return out

    enc_wT = np.zeros((FINP, H), dtype=np.float32)
    enc_wT[:FIN] = np.asarray(enc_w, np.float32).T
    encw_h = ktile_layout(enc_wT, KT)
    wp_h = ktile_layout(Wp, 2)                      # symmetric: Wp rows
    l1_h = ktile_layout(np.asarray(lin1_w, np.float32).T, 2)
    l2_h = ktile_layout(np.asarray(lin2_w, np.float32).T, 2)

    cols_h = np.zeros((128, 10), dtype=np.float32)
    cols_h[:, 0:2] = -np.asarray(ext_w, np.float32).reshape(2, 128).T
    cols_h[:, 2:4] = np.asarray(lin1_b, np.float32).reshape(2, 128).T
    cols_h[:, 4:6] = np.asarray(lin2_b, np.float32).reshape(2, 128).T
    cols_h[:, 6:8] = np.asarray(bn_gamma, np.float32).reshape(2, 128).T
    cols_h[:, 8:10] = np.asarray(bn_beta, np.float32).reshape(2, 128).T

    in_maps = []
    for c in range(NCORE):
        in_maps.append({
            "xt": xts[c], "eidx": eidxs[c], "smat": smats[c],
            "encw": encw_h, "wp": wp_h, "l1": l1_h, "l2": l2_h,
            "cols": cols_h,
        })
    return K_w, float(np.asarray(src_b).reshape(-1)[0]), in_maps


def _run(inputs, trace=False):
    K_w, src_b_val, in_maps = _host_prep(**inputs)
    nc = _build_program(list(K_w), src_b_val)
    res = run_bass_kernel_spmd(nc, in_maps, core_ids=list(range(NCORE)),
                               trace=trace)
    y = np.concatenate(
        [np.ascontiguousarray(res.results[c]["y"].T) for c in range(NCORE)], 0)
    return y.astype(np.float32), res


def kernel(**inputs):
    y, _ = _run(inputs, trace=False)
    return y


def _timed_run(inputs, n_iter=3):
    """Correctness + warm timing: jit once, device_put inputs, time execs."""
    import time as _time
    import jax
    from jax.sharding import Mesh, PartitionSpec, NamedSharding
    from jax.experimental.shard_map import shard_map
    from concourse import bass2jax, mybir as _mb

    K_w, src_b_val, in_maps = _host_prep(**inputs)
    nc = _build_program(list(K_w), src_b_val)
    bass2jax.install_neuronx_cc_hook()

    partition_name = (nc.partition_id_tensor.name
                      if nc.partition_id_tensor else None)
    in_names, out_names, out_avals, zero_outs = [], [], [], []
    for alloc in nc.m.functions[0].allocations:
        if not isinstance(alloc, _mb.MemoryLocationSet):
            continue
        name = alloc.memorylocations[0].name
        if alloc.kind == "ExternalInput":
            if name != partition_name:
                in_names.append(name)
        elif alloc.kind == "ExternalOutput":
            out_names.append(name)
            shape = tuple(alloc.tensor_shape)
            dtype = _mb.dt.np(alloc.dtype)
            out_avals.append(jax.core.ShapedArray(shape, dtype))
            zero_outs.append(np.zeros(shape, dtype))
    n_params = len(in_names)
    n_outs = len(out_avals)
    in_names_all = in_names + out_names
    if partition_name is not None:
        in_names_all.append(partition_name)
    donate = tuple(range(n_params, n_params + n_outs))

    def _body(*args):
        operands = list(args)
        if partition_name is not None:
            operands.append(bass2jax.partition_id_tensor())
        outs = bass2jax._bass_exec_p.bind(
            *operands, out_avals=tuple(out_avals),
            in_names=tuple(in_names_all), out_names=tuple(out_names),
            lowering_input_output_aliases=(),
            sim_require_finite=True, sim_require_nnan=True, nc=nc)
        return tuple(outs)

    devices = jax.devices()[:NCORE]
    mesh = Mesh(np.asarray(devices), ("core",))
    sharded = jax.jit(
        shard_map(_body, mesh=mesh,
                  in_specs=(PartitionSpec("core"),) * (n_params + n_outs),
                  out_specs=(PartitionSpec("core"),) * n_outs,
                  check_rep=False),
        donate_argnums=donate, keep_unused=True)

    sh = NamedSharding(mesh, PartitionSpec("core"))
    concat_in = [
        jax.device_put(
            np.concatenate([np.asarray(in_maps[c][n]) for c in range(NCORE)], 0),
            sh)
        for n in in_names]
    times = []
    out_arrs = None
    for it in range(n_iter):
        concat_zeros = [
            jax.device_put(np.zeros((NCORE * z.shape[0], *z.shape[1:]), z.dtype), sh)
            for z in zero_outs]
        for z in concat_zeros:
            z.block_until_ready()
        t0 = _time.perf_counter()
        out_arrs = sharded(*concat_in, *concat_zeros)
        for o in out_arrs:
            o.block_until_ready()
        times.append(_time.perf_counter() - t0)
    y_full = np.asarray(out_arrs[out_names.index("y")])
    y = y_full.reshape(NCORE, 2 * 128, NL).transpose(0, 2, 1).reshape(
        NCORE * NL, H)
    return y.astype(np.float32), times
